# revision 1
# baseline (speedup 1.0000x reference)
"""Trainium2 Bass kernel for nn_EquivariantProductBasisBlock (MACE symmetric
contraction, correlation 3), data-parallel over nodes on 8 NeuronCores.

Formulation: per (node b, channel c) row, with x = node_feats[b, c*9:(c+1)*9],
  y[b,c,(l,m)] = sum_k w_nu_l[s_b,k,c] * sum_mu U[mu,(l,m,nu,k)] * z_mu(x)
where z = [x (9), sym xx (45), sym xxx (165)] monomials (219 total), then
  out[b,d,(l,m)] = (1/sqrt(C)) sum_c wlin_l[c,d] * y[b,c,(l,m)].

Device pipeline per 512-row chunk (4 nodes x 128 channels), rows on the
matmul free axis:
  E^T[99, rows]  = U1[128,99].T @ Z1 + U2[91,99].T @ Z2          (PE, psum)
  P[99, rows]    = E^T * WT[99, s(b)*128+c]                       (DVE)
  y_b[128c, 9]   = P_b[99,128].T @ SEL[99,9]   per node           (PE: k-sum
                                                                  + transpose)
  out_b[128d, m] = WL_l[128,128].T @ y[:, m-slice]  batched       (PE)
Monomials Z1/Z2 and all weight repacks are host-side numpy. Nodes are sorted
by specie with identical per-specie slot counts on every core so one SPMD
program serves all cores (species offsets are compile-time constants).
"""

import dataclasses
import math

import numpy as np

import concourse.bass as bass
import concourse.mybir as mybir
from concourse.bass_utils import run_bass_kernel_spmd
from concourse.tile import TileContext

# ---------------- problem constants (hardcoded per spec) ----------------
N_NODES = 2048
C = 128
DIM = 9
MS = (1, 3, 5)
NSPEC = 10
NCORES = 8
NJ = 99                      # (l, m, nu, k) columns
NJP = 100                    # NJ padded even for fp32r matmul ISA rules
YW = 10                      # per-node y columns (9 + 1 pad, even)
NZ1, NZ2 = 128, 91           # z rows split: [x(9); xx(45); xxx[0:74]] | xxx[74:165]
F = 512                      # rows per chunk = 4 nodes
LOFF = (0, 1, 4)             # (l,m) column offset of l-block within the 9 y-cols
OBASE = (0, 128, 512)        # output column base of l-block

XX_IDX = [(q, r) for q in range(DIM) for r in range(q, DIM)]
XXX_IDX = [(p, q, r) for p in range(DIM) for q in range(p, DIM) for r in range(q, DIM)]
XX_POS = {qr: i for i, qr in enumerate(XX_IDX)}
COLMAP = [
    (l, m, nu, k)
    for l in range(3)
    for m in range(MS[l])
    for nu, nk in ((3, 7), (2, 3), (1, 1))
    for k in range(nk)
]
assert len(COLMAP) == NJ

F32 = mybir.dt.float32
F32R = mybir.dt.float32r
BF16 = mybir.dt.bfloat16

ZDT = BF16  # dtype of z / U / SEL / P / matmul operands
import ml_dtypes
_ZNP = ml_dtypes.bfloat16 if ZDT == BF16 else np.float32


def _mult3(p, q, r):
    if p == q == r:
        return 1.0
    if p == q or q == r or p == r:
        return 3.0
    return 6.0


def _bcast_free(ap, count, axis_elems):
    """[P, axis_elems] AP -> [P, count, axis_elems] with a step-0 middle dim."""
    a = list(list(d) for d in ap.ap)
    assert len(a) == 2 and a[1][1] == axis_elems
    return dataclasses.replace(ap, ap=[a[0], [0, count], a[1]])


def _out_ap(out_param, node0, nnodes, l, ml):
    """DRAM AP for out[node0+n, OBASE[l] + d*ml + m], traversal (d, m, n)."""
    ap = out_param[:, :]
    return dataclasses.replace(
        ap,
        offset=node0 * 1152 + OBASE[l],
        ap=[[ml, 128], [1, ml], [1152, nnodes]],
    )


# ---------------- walrus workaround: split multi-sem-waits ----------------
_MAXW = 1
_nop_ctr = [0]


def _split_waits_in_ordered(nc, ordered):
    """Walrus (this build) rejects instructions with >_MAXW sync waits: move
    excess waits onto same-engine InstNoOp carriers spliced just before."""
    for bb_name, insts in ordered.items():
        out = []
        for inst in insts:
            si = inst.sync_info
            if si is not None and len(si.on_wait) > _MAXW:
                waits = list(si.on_wait)
                keep = waits[: _MAXW]
                rest = waits[_MAXW:]
                for i in range(0, len(rest), _MAXW):
                    _nop_ctr[0] += 1
                    nop = mybir.InstNoOp(name=f"I-waitnop-{_nop_ctr[0]}")
                    nop.engine = inst.engine
                    nop.sync_info = mybir.SyncInfo(
                        on_wait=rest[i : i + _MAXW], on_update=[]
                    )
                    nc.register_instruction(nop, overwrite=True)
                    out.append(nop)
                inst.sync_info = mybir.SyncInfo(
                    on_wait=keep, on_update=list(si.on_update)
                )
            out.append(inst)
        insts[:] = out


if not getattr(TileContext, "_ant_waitsplit_patched", False):
    _orig_lower_ordered = TileContext._lower_ordered_insts

    def _patched_lower_ordered(self, ordered):
        _split_waits_in_ordered(self.nc, ordered)
        return _orig_lower_ordered(self, ordered)

    TileContext._lower_ordered_insts = _patched_lower_ordered
    TileContext._ant_waitsplit_patched = True


def _patched_drain_and_barrier(self, tick_clock, wait_clock):
    from concourse.vector_clock import ScopedClock

    drain_inst = self.nc.sync.drain()
    wait_clock.add_sem_waits(
        drain_inst.ins, ScopedClock({None: tick_clock.global_clock})
    )
    si = drain_inst.ins.sync_info
    if si is not None and len(si.on_wait) > 1:
        waits = list(si.on_wait)
        drain_inst.ins.sync_info = mybir.SyncInfo(
            on_wait=waits[:1], on_update=list(si.on_update)
        )
        for i in range(1, len(waits)):
            nop = self.nc.sync.nop(nofuse=True)
            nop.ins.sync_info = mybir.SyncInfo(on_wait=[waits[i]], on_update=[])
    self.nc.all_engine_barrier()
    assert self.sems is not None
    popped = self.nc._tile_sem_poison_stack.pop()
    assert popped is self._sem_poison
    self.nc.clear_and_free_semaphores(list(self.sems.allocated().values()))
    self.nc.all_engine_barrier()


TileContext._drain_and_barrier = _patched_drain_and_barrier


# ---------------- host-side preprocessing ----------------
def _layout(node_specie):
    """Identical per-core specie layout. Returns (slot_species, slots_per_core).

    slot_species: list of length T4 (specie of each slot, same on all cores).
    slots_per_core: int array [NCORES, T4] of original node ids (-1 = dummy).
    """
    spec = np.asarray(node_specie)
    order = np.argsort(spec, kind="stable")
    by_s = [order[spec[order] == s] for s in range(NSPEC)]
    cs = [math.ceil(len(b) / NCORES) for b in by_s]
    T = sum(cs)
    T4 = ((T + 3) // 4) * 4
    pad = T4 - T
    slot_species = []
    for s in range(NSPEC):
        slot_species += [s] * cs[s]
    slot_species += [0] * pad
    slots = -np.ones((NCORES, T4), np.int64)
    col = 0
    for s in range(NSPEC):
        nodes = by_s[s]
        for i in range(NCORES):
            take = nodes[i * cs[s] : (i + 1) * cs[s]]
            slots[i, col : col + len(take)] = take
        col += cs[s]
    return slot_species, slots


def _host_pack(inputs, slots):
    """Build per-core Z1/Z2 and shared weight blocks (all numpy, float32)."""
    x = np.ascontiguousarray(inputs["node_feats"], np.float32).reshape(N_NODES, C, DIM)
    T4 = slots.shape[1]
    R = T4 * C
    p3 = np.array([m[0] for m in XXX_IDX])
    nu3 = np.array([XX_POS[(m[1], m[2])] for m in XXX_IDX])
    q2 = np.array([m[0] for m in XX_IDX])
    r2 = np.array([m[1] for m in XX_IDX])

    z1s, z2s = [], []
    for i in range(NCORES):
        sl = slots[i]
        xs = np.zeros((T4, C, DIM), np.float32)
        valid = sl >= 0
        xs[valid] = x[sl[valid]]
        rows = xs.reshape(R, DIM)
        xx = rows[:, q2] * rows[:, r2]            # [R, 45]
        xxx = xx[:, nu3] * rows[:, p3]            # [R, 165]
        z1 = np.empty((NZ1, R), np.float32)
        z1[0:9] = rows.T
        z1[9:54] = xx.T
        z1[54:128] = xxx[:, 0:74].T
        z2 = np.ascontiguousarray(xxx[:, 74:165].T)
        z1s.append(z1)
        z2s.append(z2)

    # U [219, 99] with symmetry multiplicities, split into U1/U2 blocks
    U = np.zeros((219, NJ), np.float32)
    for j, (l, m, nu, k) in enumerate(COLMAP):
        if nu == 3:
            u3 = inputs[f"u3_l{l}"]
            for i, (p, q, r) in enumerate(XXX_IDX):
                U[54 + i, j] = _mult3(p, q, r) * u3[m, p, q, r, k]
        elif nu == 2:
            u2 = inputs[f"u2_l{l}"]
            for i, (q, r) in enumerate(XX_IDX):
                U[9 + i, j] = (1.0 if q == r else 2.0) * u2[m, q, r, k]
        else:
            u1 = inputs[f"u1_l{l}"]
            U[0:9, j] = u1[m, :, k]
    Up = np.zeros((219, NJP), np.float32)
    Up[:, :NJ] = U
    U1 = np.ascontiguousarray(Up[0:128])
    U2 = np.ascontiguousarray(Up[128:219])

    WT = np.zeros((NJP, NSPEC, C), np.float32)
    for j, (l, m, nu, k) in enumerate(COLMAP):
        WT[j] = inputs[f"w{nu}_l{l}"][:, k, :]
    WT = WT.reshape(NJP, NSPEC * C)

    SEL = np.zeros((NJP, YW), np.float32)
    for j, (l, m, nu, k) in enumerate(COLMAP):
        SEL[j, LOFF[l] + m] = 1.0

    WL = np.empty((C, 3 * C), np.float32)
    scale = 1.0 / math.sqrt(C)
    for l in range(3):
        WL[:, l * C : (l + 1) * C] = inputs[f"wlin_l{l}"] * scale

    return z1s, z2s, U1, U2, WT, SEL, WL


# ---------------- device program ----------------
def _chunk_runs(slot_species, ch):
    """Consecutive same-specie runs among the 4 nodes of chunk ch."""
    sp = slot_species[ch * 4 : (ch + 1) * 4]
    runs = []
    i = 0
    while i < 4:
        j = i
        while j < 4 and sp[j] == sp[i]:
            j += 1
        runs.append((sp[i], i, j))
        i = j
    return runs


def _build_program(slot_species, repeat=1, phase1_only=False):
    T4 = len(slot_species)
    R = T4 * C
    NCH = R // F
    nc = bass.Bass()
    Z1 = nc.declare_dram_parameter("Z1", [NZ1, R], ZDT, isOutput=False)
    Z2 = nc.declare_dram_parameter("Z2", [NZ2, R], ZDT, isOutput=False)
    U1 = nc.declare_dram_parameter("U1", [NZ1, NJP], ZDT, isOutput=False)
    U2 = nc.declare_dram_parameter("U2", [NZ2, NJP], ZDT, isOutput=False)
    WT = nc.declare_dram_parameter("WT", [NJP, NSPEC * C], F32, isOutput=False)
    SELP = nc.declare_dram_parameter("SEL", [NJP, YW], ZDT, isOutput=False)
    WL = nc.declare_dram_parameter("WL", [C, 3 * C], F32R, isOutput=False)
    OUT = nc.declare_dram_parameter("OUT", [T4, 1152], F32, isOutput=True)

    # specie runs over the whole core (nodes are specie-sorted)
    runs = []
    i = 0
    while i < T4:
        j = i
        while j < T4 and slot_species[j] == slot_species[i]:
            j += 1
        runs.append((slot_species[i], i, j))
        i = j

    GN = 12  # nodes per SELK psum/copy group

    with TileContext(nc) as tc:
        with (
            tc.tile_pool(name="wts", bufs=1) as wpool,
            tc.tile_pool(name="z", bufs=4) as zpool,
            tc.tile_pool(name="big", bufs=1) as bigpool,
            tc.tile_pool(name="yall", bufs=1) as ypool,
            tc.tile_pool(name="ostage", bufs=3) as opool,
            tc.tile_pool(name="et", bufs=4, space="PSUM") as etpool,
            tc.tile_pool(name="yps", bufs=2, space="PSUM") as ypspool,
            tc.tile_pool(name="ops", bufs=2, space="PSUM") as opspool,
        ):
            u1_t = wpool.tile([NZ1, NJP], ZDT, tag="u1")
            nc.sync.dma_start(out=u1_t[:, :], in_=U1[:, :])
            u2_t = wpool.tile([NZ2, NJP], ZDT, tag="u2")
            nc.sync.dma_start(out=u2_t[:, :], in_=U2[:, :])
            wt_t = wpool.tile([NJP, NSPEC * C], F32, tag="wt")
            nc.sync.dma_start(out=wt_t[:, :], in_=WT[:, :])
            sel_t = wpool.tile([NJP, YW], ZDT, tag="sel")
            nc.sync.dma_start(out=sel_t[:, :], in_=SELP[:, :])
            wl_t = wpool.tile([C, 3 * C], F32R, tag="wl")
            nc.sync.dma_start(out=wl_t[:, :], in_=WL[:, :])
            y_all = ypool.tile([C, T4 * YW], F32R, tag="yall")
            e_sb = bigpool.tile([NJP, R], BF16, tag="esb")
            p_sb = bigpool.tile([NJP, R], BF16, tag="psb")

            import contextlib
            loop_ctx = tc.For_i(0, repeat, 1) if repeat > 1 else contextlib.nullcontext()
            with loop_ctx:
                # phase 1: E = U.T @ Z, copied psum -> sbuf (bf16)
                for ch in range(NCH):
                    z1 = zpool.tile([NZ1, F], ZDT, tag="z1")
                    nc.sync.dma_start(out=z1[:, :], in_=Z1[:, ch * F : (ch + 1) * F])
                    z2 = zpool.tile([NZ2, F], ZDT, tag="z2")
                    nc.sync.dma_start(out=z2[:, :], in_=Z2[:, ch * F : (ch + 1) * F])
                    et = etpool.tile([NJP, F], F32, tag="et")
                    nc.tensor.matmul(et[:, :], u1_t[:, :], z1[:, :], start=True, stop=False)
                    nc.tensor.matmul(et[:, :], u2_t[:, :], z2[:, :], start=False, stop=True)
                    nc.scalar.copy(out=e_sb[:, ch * F : (ch + 1) * F], in_=et[:, :])
                if phase1_only:
                    otx = opool.tile([NJP, F], F32, tag="otx")
                    nc.scalar.copy(out=otx[:, :], in_=e_sb[:, 0:F])
                    nc.sync.dma_start(out=OUT[0:NJP, 0:F], in_=otx[:, :])
                # phase 2: species multiply, one DVE op per specie run
                for s, i0, i1 in (runs if not phase1_only else []):
                    n = i1 - i0
                    w_ap = _bcast_free(wt_t[:, s * C : (s + 1) * C], n, C)
                    nc.vector.tensor_mul(
                        p_sb[:, i0 * C : i1 * C].rearrange("j (n c) -> j n c", c=C),
                        e_sb[:, i0 * C : i1 * C].rearrange("j (n c) -> j n c", c=C),
                        w_ap,
                    )
                # phase 3: per-node SELK (k-sum + transpose), grouped psum/copies
                for g0 in (range(0, T4, GN) if not phase1_only else []):
                    g1 = min(g0 + GN, T4)
                    yps = ypspool.tile([C, GN * YW], F32, tag="yps")
                    for t in range(g0, g1):
                        nc.tensor.matmul(
                            yps[:, (t - g0) * YW : (t - g0 + 1) * YW],
                            p_sb[:, t * C : (t + 1) * C],
                            sel_t[:, :],
                            start=True,
                            stop=True,
                        )
                    nc.scalar.copy(
                        out=y_all[:, g0 * YW : g1 * YW],
                        in_=yps[:, : (g1 - g0) * YW],
                    )
                # phase 4: wlin
                yv = y_all[:, :].rearrange("c (n j) -> c j n", j=YW)
                for l in (range(3) if not phase1_only else []):
                    ml = MS[l]
                    for g0 in range(0, T4, 32):
                        gn = min(32, T4 - g0)
                        ops = opspool.tile([C, 32 * 5], F32, tag="ops")
                        nc.tensor.matmul(
                            ops[:, : ml * gn],
                            wl_t[:, l * C : (l + 1) * C],
                            yv[:, LOFF[l] : LOFF[l] + ml, g0 : g0 + gn],
                            start=True,
                            stop=True,
                        )
                        ot = opool.tile([C, 32 * 5], F32, tag="ot")
                        nc.scalar.copy(out=ot[:, : ml * gn], in_=ops[:, : ml * gn])
                        nc.sync.dma_start(
                            out=_out_ap(OUT, g0, gn, l, ml),
                            in_=ot[:, : ml * gn].rearrange("d (m n) -> d m n", n=gn),
                        )
    return nc


_CACHE = {}


def _get_program(slot_species, repeat=1, phase1_only=False):
    key = (tuple(slot_species), repeat, phase1_only)
    if key not in _CACHE:
        _CACHE[key] = _build_program(slot_species, repeat, phase1_only)
    return _CACHE[key]


def make_in_maps(inputs):
    """Host prep shared by kernel() and the timing harness."""
    slot_species, slots = _layout(inputs["node_specie"])
    z1s, z2s, U1, U2, WT, SEL, WL = _host_pack(inputs, slots)
    U1 = U1.astype(_ZNP)
    U2 = U2.astype(_ZNP)
    SEL = SEL.astype(_ZNP)
    in_maps = [
        {"Z1": z1s[i].astype(_ZNP), "Z2": z2s[i].astype(_ZNP), "U1": U1, "U2": U2,
         "WT": WT, "SEL": SEL, "WL": WL}
        for i in range(NCORES)
    ]
    return slot_species, slots, in_maps


def assemble(results, slots):
    out = np.zeros((N_NODES, 1152), np.float32)
    for i in range(NCORES):
        co = results[i]["OUT"]
        sl = slots[i]
        valid = sl >= 0
        out[sl[valid]] = co[valid]
    return out


def kernel(**inputs):
    inputs = {k: np.asarray(v) for k, v in inputs.items()}
    slot_species, slots, in_maps = make_in_maps(inputs)
    nc = _get_program(slot_species)
    res = run_bass_kernel_spmd(nc, in_maps, list(range(NCORES)))
    return assemble(res.results, slots)



# revision 3
# speedup vs baseline: 24.3979x; 24.3979x over previous
"""Trainium2 Bass kernel for nn_EquivariantProductBasisBlock (MACE symmetric
contraction, correlation 3), data-parallel over nodes on 8 NeuronCores.

Formulation: per (node b, channel c) row, with x = node_feats[b, c*9:(c+1)*9],
  y[b,c,(l,m)] = sum_k w_nu_l[s_b,k,c] * sum_mu U[mu,(l,m,nu,k)] * z_mu(x)
where z = [x (9), sym xx (45), sym xxx (165)] monomials (219 total), then
  out[b,d,(l,m)] = (1/sqrt(C)) sum_c wlin_l[c,d] * y[b,c,(l,m)].

Device pipeline (per core, T4 nodes, R = T4*128 (node,channel) columns):
  - Z1 [128,R] / Z2 [91,R] bf16 monomial blocks stay RESIDENT in SBUF,
    loaded each iteration by a few multi-MB DMAs (8KB+ descriptors).
  - per 512-col chunk (4 nodes): et[100,512](psum) = U1.T@Z1c + U2.T@Z2c;
    DVE: p = et * WT[specie] (bf16, sbuf); per node SELK matmul
    y_n[128c, 10] = p_n[100,128].T @ SEL[100,10] accumulated into y_all.
  - per 128-node group: 9 matmuls out_lm[n,d] = y[c,n].T @ WL_l[c,d] put
    NODES on partitions; strided psum->sbuf copies build OUTT[n,1152] so
    the store is one contiguous-row DMA per group (4.6KB descriptors).
Nodes are sorted by specie with identical per-specie slot counts on every
core so one SPMD program serves all cores.
"""

import dataclasses
import math

import numpy as np

import concourse.bass as bass
import concourse.mybir as mybir
from concourse.bass_utils import run_bass_kernel_spmd
from concourse.tile import TileContext

# ---------------- problem constants (hardcoded per spec) ----------------
N_NODES = 2048
C = 128
DIM = 9
MS = (1, 3, 5)
NSPEC = 10
NCORES = 8
NJ = 99                      # (l, m, nu, k) columns
NJP = 100                    # padded
YW = 10                      # per-node y columns (9 + 1 pad)
NZ1, NZ2 = 128, 91           # z rows split: [x(9); xx(45); xxx[0:74]] | xxx[74:165]
F = 512                      # columns per chunk = 4 nodes
LAG = 2                      # chunks between et production and SELK consumption
GN = 16                      # nodes per yps psum group (4 chunks)
PIECE = 8                    # chunks per Z DMA piece
LOFF = (0, 1, 4)             # (l,m) column offset of l-block within the 10 y-cols
OBASE = (0, 128, 512)        # output column base of l-block

XX_IDX = [(q, r) for q in range(DIM) for r in range(q, DIM)]
XXX_IDX = [(p, q, r) for p in range(DIM) for q in range(p, DIM) for r in range(q, DIM)]
XX_POS = {qr: i for i, qr in enumerate(XX_IDX)}
COLMAP = [
    (l, m, nu, k)
    for l in range(3)
    for m in range(MS[l])
    for nu, nk in ((3, 7), (2, 3), (1, 1))
    for k in range(nk)
]
assert len(COLMAP) == NJ

F32 = mybir.dt.float32
BF16 = mybir.dt.bfloat16

import ml_dtypes
_ZNP = ml_dtypes.bfloat16


def _mult3(p, q, r):
    if p == q == r:
        return 1.0
    if p == q or q == r or p == r:
        return 3.0
    return 6.0


def _bcast_free(ap, count, axis_elems):
    """[P, axis_elems] AP -> [P, count, axis_elems] with a step-0 middle dim."""
    a = list(list(d) for d in ap.ap)
    assert len(a) == 2 and a[1][1] == axis_elems
    return dataclasses.replace(ap, ap=[a[0], [0, count], a[1]])


# ---------------- walrus workaround: split multi-sem-waits ----------------
_MAXW = 1
_nop_ctr = [0]


def _split_waits_in_ordered(nc, ordered):
    """Walrus (this build) rejects instructions with >_MAXW sync waits: move
    excess waits onto same-engine InstNoOp carriers spliced just before."""
    for bb_name, insts in ordered.items():
        out = []
        for inst in insts:
            si = inst.sync_info
            if si is not None and len(si.on_wait) > _MAXW:
                waits = list(si.on_wait)
                keep = waits[: _MAXW]
                rest = waits[_MAXW:]
                for i in range(0, len(rest), _MAXW):
                    _nop_ctr[0] += 1
                    nop = mybir.InstNoOp(name=f"I-waitnop-{_nop_ctr[0]}")
                    nop.engine = inst.engine
                    nop.sync_info = mybir.SyncInfo(
                        on_wait=rest[i : i + _MAXW], on_update=[]
                    )
                    nc.register_instruction(nop, overwrite=True)
                    out.append(nop)
                inst.sync_info = mybir.SyncInfo(
                    on_wait=keep, on_update=list(si.on_update)
                )
            out.append(inst)
        insts[:] = out


if not getattr(TileContext, "_ant_waitsplit_patched", False):
    _orig_lower_ordered = TileContext._lower_ordered_insts

    def _patched_lower_ordered(self, ordered):
        _split_waits_in_ordered(self.nc, ordered)
        return _orig_lower_ordered(self, ordered)

    TileContext._lower_ordered_insts = _patched_lower_ordered
    TileContext._ant_waitsplit_patched = True


def _patched_drain_and_barrier(self, tick_clock, wait_clock):
    from concourse.vector_clock import ScopedClock

    drain_inst = self.nc.sync.drain()
    wait_clock.add_sem_waits(
        drain_inst.ins, ScopedClock({None: tick_clock.global_clock})
    )
    si = drain_inst.ins.sync_info
    if si is not None and len(si.on_wait) > 1:
        waits = list(si.on_wait)
        drain_inst.ins.sync_info = mybir.SyncInfo(
            on_wait=waits[:1], on_update=list(si.on_update)
        )
        for i in range(1, len(waits)):
            nop = self.nc.sync.nop(nofuse=True)
            nop.ins.sync_info = mybir.SyncInfo(on_wait=[waits[i]], on_update=[])
    self.nc.all_engine_barrier()
    assert self.sems is not None
    popped = self.nc._tile_sem_poison_stack.pop()
    assert popped is self._sem_poison
    self.nc.clear_and_free_semaphores(list(self.sems.allocated().values()))
    self.nc.all_engine_barrier()


TileContext._drain_and_barrier = _patched_drain_and_barrier


# ---------------- host-side preprocessing ----------------
def _layout(node_specie):
    """Identical per-core specie layout. Returns (slot_species, slots_per_core)."""
    spec = np.asarray(node_specie)
    order = np.argsort(spec, kind="stable")
    by_s = [order[spec[order] == s] for s in range(NSPEC)]
    cs = [math.ceil(len(b) / NCORES) for b in by_s]
    T = sum(cs)
    T4 = ((T + 3) // 4) * 4
    pad = T4 - T
    slot_species = []
    for s in range(NSPEC):
        slot_species += [s] * cs[s]
    slot_species += [0] * pad
    slots = -np.ones((NCORES, T4), np.int64)
    col = 0
    for s in range(NSPEC):
        nodes = by_s[s]
        for i in range(NCORES):
            take = nodes[i * cs[s] : (i + 1) * cs[s]]
            slots[i, col : col + len(take)] = take
        col += cs[s]
    return slot_species, slots


def _host_pack(inputs, slots):
    """Build per-core Z1/Z2 and shared weight blocks."""
    x = np.ascontiguousarray(inputs["node_feats"], np.float32).reshape(N_NODES, C, DIM)
    T4 = slots.shape[1]
    R = T4 * C
    p3 = np.array([m[0] for m in XXX_IDX])
    nu3 = np.array([XX_POS[(m[1], m[2])] for m in XXX_IDX])
    q2 = np.array([m[0] for m in XX_IDX])
    r2 = np.array([m[1] for m in XX_IDX])

    z1s, z2s = [], []
    for i in range(NCORES):
        sl = slots[i]
        xs = np.zeros((T4, C, DIM), np.float32)
        valid = sl >= 0
        xs[valid] = x[sl[valid]]
        rows = xs.reshape(R, DIM)
        xx = rows[:, q2] * rows[:, r2]            # [R, 45]
        xxx = xx[:, nu3] * rows[:, p3]            # [R, 165]
        z1 = np.empty((NZ1, R), np.float32)
        z1[0:9] = rows.T
        z1[9:54] = xx.T
        z1[54:128] = xxx[:, 0:74].T
        z2 = np.ascontiguousarray(xxx[:, 74:165].T)
        z1s.append(z1.astype(_ZNP))
        z2s.append(z2.astype(_ZNP))

    # U [219, 99] with symmetry multiplicities, split into U1/U2 blocks
    U = np.zeros((219, NJ), np.float32)
    for j, (l, m, nu, k) in enumerate(COLMAP):
        if nu == 3:
            u3 = inputs[f"u3_l{l}"]
            for i, (p, q, r) in enumerate(XXX_IDX):
                U[54 + i, j] = _mult3(p, q, r) * u3[m, p, q, r, k]
        elif nu == 2:
            u2 = inputs[f"u2_l{l}"]
            for i, (q, r) in enumerate(XX_IDX):
                U[9 + i, j] = (1.0 if q == r else 2.0) * u2[m, q, r, k]
        else:
            u1 = inputs[f"u1_l{l}"]
            U[0:9, j] = u1[m, :, k]
    Up = np.zeros((219, NJP), np.float32)
    Up[:, :NJ] = U
    U1 = np.ascontiguousarray(Up[0:128]).astype(_ZNP)
    U2 = np.ascontiguousarray(Up[128:219]).astype(_ZNP)

    WT = np.zeros((NJP, NSPEC, C), np.float32)
    for j, (l, m, nu, k) in enumerate(COLMAP):
        WT[j] = inputs[f"w{nu}_l{l}"][:, k, :]
    WT = WT.reshape(NJP, NSPEC * C)

    SEL = np.zeros((NJP, YW), np.float32)
    for j, (l, m, nu, k) in enumerate(COLMAP):
        SEL[j, LOFF[l] + m] = 1.0
    SEL = SEL.astype(_ZNP)

    WL = np.empty((C, 3 * C), np.float32)
    scale = 1.0 / math.sqrt(C)
    for l in range(3):
        WL[:, l * C : (l + 1) * C] = inputs[f"wlin_l{l}"] * scale
    WL = WL.astype(_ZNP)

    return z1s, z2s, U1, U2, WT, SEL, WL


# ---------------- device program ----------------
def _build_program(slot_species, repeat=1):
    T4 = len(slot_species)
    R = T4 * C
    NCH = T4 // 4
    NGF = T4 // 128              # full 128-node output groups
    nc = bass.Bass()
    Z1 = nc.declare_dram_parameter("Z1", [NZ1, R], BF16, isOutput=False)
    Z2 = nc.declare_dram_parameter("Z2", [NZ2, R], BF16, isOutput=False)
    U1 = nc.declare_dram_parameter("U1", [NZ1, NJP], BF16, isOutput=False)
    U2 = nc.declare_dram_parameter("U2", [NZ2, NJP], BF16, isOutput=False)
    WT = nc.declare_dram_parameter("WT", [NJP, NSPEC * C], F32, isOutput=False)
    SELP = nc.declare_dram_parameter("SEL", [NJP, YW], BF16, isOutput=False)
    WL = nc.declare_dram_parameter("WL", [C, 3 * C], BF16, isOutput=False)
    OUT = nc.declare_dram_parameter("OUT", [T4, 1152], F32, isOutput=True)

    with TileContext(nc) as tc:
        with (
            tc.tile_pool(name="wts", bufs=1) as wpool,
            tc.tile_pool(name="zres", bufs=1) as zpool,
            tc.tile_pool(name="p", bufs=4) as ppool,
            tc.tile_pool(name="yall", bufs=1) as ypool,
            tc.tile_pool(name="ostage", bufs=2) as opool,
            tc.tile_pool(name="et", bufs=4, space="PSUM") as etpool,
            tc.tile_pool(name="yps", bufs=2, space="PSUM") as ypspool,
            tc.tile_pool(name="ops", bufs=2, space="PSUM") as opspool,
        ):
            u1_t = wpool.tile([NZ1, NJP], BF16, tag="u1")
            nc.sync.dma_start(out=u1_t[:, :], in_=U1[:, :])
            u2_t = wpool.tile([NZ2, NJP], BF16, tag="u2")
            nc.sync.dma_start(out=u2_t[:, :], in_=U2[:, :])
            wt_t = wpool.tile([NJP, NSPEC * C], F32, tag="wt")
            nc.sync.dma_start(out=wt_t[:, :], in_=WT[:, :])
            sel_t = wpool.tile([NJP, YW], BF16, tag="sel")
            nc.sync.dma_start(out=sel_t[:, :], in_=SELP[:, :])
            wl_t = wpool.tile([C, 3 * C], BF16, tag="wl")
            nc.sync.dma_start(out=wl_t[:, :], in_=WL[:, :])

            z1_t = zpool.tile([NZ1, R], BF16, tag="z1")
            z2_t = zpool.tile([NZ2, R], BF16, tag="z2")
            y_all = ypool.tile([C, T4 * YW], BF16, tag="yall")
            yv = y_all[:, :].rearrange("c (n j) -> c j n", j=YW)

            import contextlib
            loop_ctx = tc.For_i(0, repeat, 1) if repeat > 1 else contextlib.nullcontext()
            with loop_ctx:
                # Z loads: multi-chunk pieces, interleaved z1/z2 in need order
                for p0 in range(0, NCH, PIECE):
                    c0 = p0 * F
                    c1 = min((p0 + PIECE) * F, R)
                    nc.sync.dma_start(out=z1_t[:, c0:c1], in_=Z1[:, c0:c1])
                    nc.sync.dma_start(out=z2_t[:, c0:c1], in_=Z2[:, c0:c1])

                p_tiles = {}
                yps_box = [None]

                def emit_chunk(k):
                    c0 = k * F
                    et = etpool.tile([NJP, F], F32, tag="et")
                    nc.tensor.matmul(et[:, :], u1_t[:, :], z1_t[:, c0:c0 + F],
                                     start=True, stop=False)
                    nc.tensor.matmul(et[:, :], u2_t[:, :], z2_t[:, c0:c0 + F],
                                     start=False, stop=True)
                    p_t = ppool.tile([NJP, F], BF16, tag="p")
                    p_tiles[k] = p_t
                    # specie runs among this chunk's 4 nodes
                    i = 0
                    while i < 4:
                        j = i
                        s = slot_species[4 * k + i]
                        while j < 4 and slot_species[4 * k + j] == s:
                            j += 1
                        w_ap = _bcast_free(wt_t[:, s * C:(s + 1) * C], j - i, C)
                        nc.vector.tensor_mul(
                            p_t[:, i * C:j * C].rearrange("j (n c) -> j n c", c=C),
                            et[:, i * C:j * C].rearrange("j (n c) -> j n c", c=C),
                            w_ap,
                        )
                        i = j

                def emit_selk(k):
                    p_t = p_tiles.pop(k)
                    for jj in range(4):
                        t = 4 * k + jj
                        if t % GN == 0:
                            yps_box[0] = ypspool.tile([C, GN * YW], F32,
                                                      name="yps", tag="yps")
                        yps = yps_box[0]
                        nc.tensor.matmul(
                            yps[:, (t % GN) * YW:(t % GN + 1) * YW],
                            p_t[:, jj * C:(jj + 1) * C],
                            sel_t[:, :],
                            start=True, stop=True,
                        )
                        if t % GN == GN - 1 or t == T4 - 1:
                            g0 = (t // GN) * GN
                            cnt = t - g0 + 1
                            nc.scalar.copy(
                                out=y_all[:, g0 * YW:(g0 + cnt) * YW],
                                in_=yps[:, :cnt * YW],
                            )

                def emit_phase4(h):
                    n0 = h * 128
                    gn = min(128, T4 - n0)
                    ot = opool.tile([128, 1152], F32, tag="ot")
                    for l in range(3):
                        ml = MS[l]
                        otv = ot[0:gn, OBASE[l]:OBASE[l] + 128 * ml].rearrange(
                            "n (d x) -> n x d", x=ml)
                        for m in range(ml):
                            ops = opspool.tile([128, C], F32, tag="ops")
                            nc.tensor.matmul(
                                ops[0:gn, :],
                                yv[:, LOFF[l] + m, n0:n0 + gn],
                                wl_t[:, l * C:(l + 1) * C],
                                start=True, stop=True,
                            )
                            nc.scalar.copy(out=otv[:, m, :], in_=ops[0:gn, :])
                    nc.sync.dma_start(out=OUT[n0:n0 + gn, :], in_=ot[0:gn, :])

                done_h = [0]

                def maybe_phase4(k_done):
                    # group h complete after selk of chunk (h+1)*32 - 1
                    while done_h[0] < NGF and k_done == (done_h[0] + 1) * 32 - 1:
                        emit_phase4(done_h[0])
                        done_h[0] += 1

                for k in range(NCH):
                    emit_chunk(k)
                    if k >= LAG:
                        emit_selk(k - LAG)
                        maybe_phase4(k - LAG)
                for k in range(max(0, NCH - LAG), NCH):
                    emit_selk(k)
                    maybe_phase4(k)
                for h in range(done_h[0], (T4 + 127) // 128):
                    emit_phase4(h)
    return nc


_CACHE = {}


def _get_program(slot_species, repeat=1):
    key = (tuple(slot_species), repeat)
    if key not in _CACHE:
        _CACHE[key] = _build_program(slot_species, repeat)
    return _CACHE[key]


def make_in_maps(inputs):
    """Host prep shared by kernel() and the timing harness."""
    slot_species, slots = _layout(inputs["node_specie"])
    z1s, z2s, U1, U2, WT, SEL, WL = _host_pack(inputs, slots)
    in_maps = [
        {"Z1": z1s[i], "Z2": z2s[i], "U1": U1, "U2": U2,
         "WT": WT, "SEL": SEL, "WL": WL}
        for i in range(NCORES)
    ]
    return slot_species, slots, in_maps


def assemble(results, slots):
    out = np.zeros((N_NODES, 1152), np.float32)
    for i in range(NCORES):
        co = results[i]["OUT"]
        sl = slots[i]
        valid = sl >= 0
        out[sl[valid]] = co[valid]
    return out


def kernel(**inputs):
    inputs = {k: np.asarray(v) for k, v in inputs.items()}
    slot_species, slots, in_maps = make_in_maps(inputs)
    nc = _get_program(slot_species)
    res = run_bass_kernel_spmd(nc, in_maps, list(range(NCORES)))
    return assemble(res.results, slots)


# revision 7
# speedup vs baseline: 27.3324x; 1.1203x over previous
"""Trainium2 Bass kernel for nn_EquivariantProductBasisBlock (MACE symmetric
contraction, correlation 3), data-parallel over nodes on 8 NeuronCores.

Formulation: per (node b, channel c) row, with x = node_feats[b, c*9:(c+1)*9],
  y[b,c,(l,m)] = sum_k w_nu_l[s_b,k,c] * sum_mu U[mu,(l,m,nu,k)] * z_mu(x)
where z = [x (9), sym xx (45), sym xxx (165)] monomials (219 total), then
  out[b,d,(l,m)] = (1/sqrt(C)) sum_c wlin_l[c,d] * y[b,c,(l,m)].

Device pipeline (per core, T4 nodes, R = T4*128 (node,channel) columns):
  - Z1 [128,R] / Z2 [91,R] bf16 monomial blocks stay RESIDENT in SBUF,
    loaded each iteration by a few multi-MB DMAs (8KB+ descriptors).
  - per 512-col chunk (4 nodes): et[100,512](psum) = U1.T@Z1c + U2.T@Z2c;
    DVE: p = et * WT[specie] (bf16, sbuf); per node SELK matmul
    y_n[128c, 10] = p_n[100,128].T @ SEL[100,10] accumulated into y_all.
  - per 128-node group: 9 matmuls out_lm[n,d] = y[c,n].T @ WL_l[c,d] put
    NODES on partitions; strided psum->sbuf copies build OUTT[n,1152] so
    the store is one contiguous-row DMA per group (4.6KB descriptors).
Nodes are sorted by specie with identical per-specie slot counts on every
core so one SPMD program serves all cores.
"""

import dataclasses
import math

import numpy as np

import concourse.bass as bass
import concourse.mybir as mybir
from concourse.bass_utils import run_bass_kernel_spmd
from concourse.tile import TileContext

# ---------------- problem constants (hardcoded per spec) ----------------
N_NODES = 2048
C = 128
DIM = 9
MS = (1, 3, 5)
NSPEC = 10
NCORES = 8
NJ = 99                      # (l, m, nu, k) columns
NJP = 100                    # padded
YW = 10                      # per-node y columns (9 + 1 pad)
NZ1, NZ2 = 128, 91           # z rows split: [x(9); xx(45); xxx[0:74]] | xxx[74:165]
F = 512                      # columns per chunk = 4 nodes
LAG = 4                      # chunks between et production and SELK consumption
GN = 32                      # nodes per yps psum group (8 chunks)
PIECE = 8                    # chunks per Z DMA piece
LOFF = (0, 1, 4)             # (l,m) column offset of l-block within the 10 y-cols
OBASE = (0, 128, 512)        # output column base of l-block

XX_IDX = [(q, r) for q in range(DIM) for r in range(q, DIM)]
XXX_IDX = [(p, q, r) for p in range(DIM) for q in range(p, DIM) for r in range(q, DIM)]
XX_POS = {qr: i for i, qr in enumerate(XX_IDX)}
COLMAP = [
    (l, m, nu, k)
    for l in range(3)
    for m in range(MS[l])
    for nu, nk in ((3, 7), (2, 3), (1, 1))
    for k in range(nk)
]
assert len(COLMAP) == NJ

F32 = mybir.dt.float32
BF16 = mybir.dt.bfloat16

import ml_dtypes
_ZNP = ml_dtypes.bfloat16


def _mult3(p, q, r):
    if p == q == r:
        return 1.0
    if p == q or q == r or p == r:
        return 3.0
    return 6.0


def _bcast_free(ap, count, axis_elems):
    """[P, axis_elems] AP -> [P, count, axis_elems] with a step-0 middle dim."""
    a = list(list(d) for d in ap.ap)
    assert len(a) == 2 and a[1][1] == axis_elems
    return dataclasses.replace(ap, ap=[a[0], [0, count], a[1]])


# ---------------- walrus workaround: split multi-sem-waits ----------------
_MAXW = 1
_nop_ctr = [0]


def _split_waits_in_ordered(nc, ordered):
    """Walrus (this build) rejects instructions with >_MAXW sync waits: move
    excess waits onto same-engine InstNoOp carriers spliced just before."""
    for bb_name, insts in ordered.items():
        out = []
        for inst in insts:
            si = inst.sync_info
            if si is not None and len(si.on_wait) > _MAXW:
                waits = list(si.on_wait)
                keep = waits[: _MAXW]
                rest = waits[_MAXW:]
                for i in range(0, len(rest), _MAXW):
                    _nop_ctr[0] += 1
                    nop = mybir.InstNoOp(name=f"I-waitnop-{_nop_ctr[0]}")
                    nop.engine = inst.engine
                    nop.sync_info = mybir.SyncInfo(
                        on_wait=rest[i : i + _MAXW], on_update=[]
                    )
                    nc.register_instruction(nop, overwrite=True)
                    out.append(nop)
                inst.sync_info = mybir.SyncInfo(
                    on_wait=keep, on_update=list(si.on_update)
                )
            out.append(inst)
        insts[:] = out


if not getattr(TileContext, "_ant_waitsplit_patched", False):
    _orig_lower_ordered = TileContext._lower_ordered_insts

    def _patched_lower_ordered(self, ordered):
        _split_waits_in_ordered(self.nc, ordered)
        return _orig_lower_ordered(self, ordered)

    TileContext._lower_ordered_insts = _patched_lower_ordered
    TileContext._ant_waitsplit_patched = True


def _patched_drain_and_barrier(self, tick_clock, wait_clock):
    from concourse.vector_clock import ScopedClock

    drain_inst = self.nc.sync.drain()
    wait_clock.add_sem_waits(
        drain_inst.ins, ScopedClock({None: tick_clock.global_clock})
    )
    si = drain_inst.ins.sync_info
    if si is not None and len(si.on_wait) > 1:
        waits = list(si.on_wait)
        drain_inst.ins.sync_info = mybir.SyncInfo(
            on_wait=waits[:1], on_update=list(si.on_update)
        )
        for i in range(1, len(waits)):
            nop = self.nc.sync.nop(nofuse=True)
            nop.ins.sync_info = mybir.SyncInfo(on_wait=[waits[i]], on_update=[])
    self.nc.all_engine_barrier()
    assert self.sems is not None
    popped = self.nc._tile_sem_poison_stack.pop()
    assert popped is self._sem_poison
    self.nc.clear_and_free_semaphores(list(self.sems.allocated().values()))
    self.nc.all_engine_barrier()


TileContext._drain_and_barrier = _patched_drain_and_barrier


# ---------------- host-side preprocessing ----------------
def _layout(node_specie):
    """Identical per-core specie layout. Returns (slot_species, slots_per_core)."""
    spec = np.asarray(node_specie)
    order = np.argsort(spec, kind="stable")
    by_s = [order[spec[order] == s] for s in range(NSPEC)]
    cs = [math.ceil(len(b) / NCORES) for b in by_s]
    T = sum(cs)
    T4 = ((T + 3) // 4) * 4
    pad = T4 - T
    slot_species = []
    for s in range(NSPEC):
        slot_species += [s] * cs[s]
    slot_species += [0] * pad
    slots = -np.ones((NCORES, T4), np.int64)
    col = 0
    for s in range(NSPEC):
        nodes = by_s[s]
        for i in range(NCORES):
            take = nodes[i * cs[s] : (i + 1) * cs[s]]
            slots[i, col : col + len(take)] = take
        col += cs[s]
    return slot_species, slots


def _host_pack(inputs, slots):
    """Build per-core Z1/Z2 and shared weight blocks."""
    x = np.ascontiguousarray(inputs["node_feats"], np.float32).reshape(N_NODES, C, DIM)
    T4 = slots.shape[1]
    R = T4 * C
    p3 = np.array([m[0] for m in XXX_IDX])
    nu3 = np.array([XX_POS[(m[1], m[2])] for m in XXX_IDX])
    q2 = np.array([m[0] for m in XX_IDX])
    r2 = np.array([m[1] for m in XX_IDX])

    z1s, z2s = [], []
    for i in range(NCORES):
        sl = slots[i]
        xs = np.zeros((T4, C, DIM), np.float32)
        valid = sl >= 0
        xs[valid] = x[sl[valid]]
        rows = xs.reshape(R, DIM)
        xx = rows[:, q2] * rows[:, r2]            # [R, 45]
        xxx = xx[:, nu3] * rows[:, p3]            # [R, 165]
        z1 = np.empty((NZ1, R), np.float32)
        z1[0:9] = rows.T
        z1[9:54] = xx.T
        z1[54:128] = xxx[:, 0:74].T
        z2 = np.ascontiguousarray(xxx[:, 74:165].T)
        z1s.append(z1.astype(_ZNP))
        z2s.append(z2.astype(_ZNP))

    # U [219, 99] with symmetry multiplicities, split into U1/U2 blocks
    U = np.zeros((219, NJ), np.float32)
    for j, (l, m, nu, k) in enumerate(COLMAP):
        if nu == 3:
            u3 = inputs[f"u3_l{l}"]
            for i, (p, q, r) in enumerate(XXX_IDX):
                U[54 + i, j] = _mult3(p, q, r) * u3[m, p, q, r, k]
        elif nu == 2:
            u2 = inputs[f"u2_l{l}"]
            for i, (q, r) in enumerate(XX_IDX):
                U[9 + i, j] = (1.0 if q == r else 2.0) * u2[m, q, r, k]
        else:
            u1 = inputs[f"u1_l{l}"]
            U[0:9, j] = u1[m, :, k]
    Up = np.zeros((219, NJP), np.float32)
    Up[:, :NJ] = U
    U1 = np.ascontiguousarray(Up[0:128]).astype(_ZNP)
    U2 = np.ascontiguousarray(Up[128:219]).astype(_ZNP)

    WT = np.zeros((NJP, NSPEC, C), np.float32)
    for j, (l, m, nu, k) in enumerate(COLMAP):
        WT[j] = inputs[f"w{nu}_l{l}"][:, k, :]
    WT = WT.reshape(NJP, NSPEC * C)

    SEL = np.zeros((NJP, YW), np.float32)
    for j, (l, m, nu, k) in enumerate(COLMAP):
        SEL[j, LOFF[l] + m] = 1.0
    SEL = SEL.astype(_ZNP)

    WL = np.empty((C, 3 * C), np.float32)
    scale = 1.0 / math.sqrt(C)
    for l in range(3):
        WL[:, l * C : (l + 1) * C] = inputs[f"wlin_l{l}"] * scale
    WL = WL.astype(_ZNP)

    return z1s, z2s, U1, U2, WT, SEL, WL


# ---------------- device program ----------------
def _build_program(slot_species, repeat=1):
    T4 = len(slot_species)
    R = T4 * C
    NCH = T4 // 4
    NGF = T4 // 128              # full 128-node output groups
    nc = bass.Bass()
    Z1 = nc.declare_dram_parameter("Z1", [NZ1, R], BF16, isOutput=False)
    Z2 = nc.declare_dram_parameter("Z2", [NZ2, R], BF16, isOutput=False)
    U1 = nc.declare_dram_parameter("U1", [NZ1, NJP], BF16, isOutput=False)
    U2 = nc.declare_dram_parameter("U2", [NZ2, NJP], BF16, isOutput=False)
    WT = nc.declare_dram_parameter("WT", [NJP, NSPEC * C], F32, isOutput=False)
    SELP = nc.declare_dram_parameter("SEL", [NJP, YW], BF16, isOutput=False)
    WL = nc.declare_dram_parameter("WL", [C, 3 * C], BF16, isOutput=False)
    OUT = nc.declare_dram_parameter("OUT", [T4, 1152], F32, isOutput=True)

    with TileContext(nc) as tc:
        with (
            tc.tile_pool(name="wts", bufs=1) as wpool,
            tc.tile_pool(name="zres", bufs=1) as zpool,
            tc.tile_pool(name="p", bufs=6) as ppool,
            tc.tile_pool(name="yall", bufs=1) as ypool,
            tc.tile_pool(name="ostage", bufs=2) as opool,
            tc.tile_pool(name="et", bufs=5, space="PSUM") as etpool,
            tc.tile_pool(name="yps", bufs=1, space="PSUM") as ypspool,
            tc.tile_pool(name="ops", bufs=2, space="PSUM") as opspool,
        ):
            z1_t = zpool.tile([NZ1, R], BF16, tag="z1")
            z2_t = zpool.tile([NZ2, R], BF16, tag="z2")
            u1_t = wpool.tile([NZ1, NJP], BF16, tag="u1")
            nc.sync.dma_start(out=u1_t[:, :], in_=U1[:, :])
            u2_t = wpool.tile([NZ2, NJP], BF16, tag="u2")
            nc.sync.dma_start(out=u2_t[:, :], in_=U2[:, :])
            wt_t = wpool.tile([NJP, NSPEC * C], F32, tag="wt")
            nc.sync.dma_start(out=wt_t[:, :], in_=WT[:, :])
            sel_t = wpool.tile([NJP, YW], BF16, tag="sel")
            nc.sync.dma_start(out=sel_t[:, :], in_=SELP[:, :])
            wl_t = wpool.tile([C, 3 * C], BF16, tag="wl")
            nc.sync.dma_start(out=wl_t[:, :], in_=WL[:, :])
            y_all = ypool.tile([C, T4 * YW], BF16, tag="yall")
            yv = y_all[:, :].rearrange("c (n j) -> c j n", j=YW)

            import contextlib
            loop_ctx = tc.For_i(0, repeat, 1) if repeat > 1 else contextlib.nullcontext()
            with loop_ctx:
                # Z loads: multi-chunk pieces, interleaved z1/z2 in need order
                for p0 in range(0, NCH, PIECE):
                    c0 = p0 * F
                    c1 = min((p0 + PIECE) * F, R)
                    nc.sync.dma_start(out=z1_t[:, c0:c1], in_=Z1[:, c0:c1])
                    nc.sync.dma_start(out=z2_t[:, c0:c1], in_=Z2[:, c0:c1])

                p_tiles = {}
                yps_box = [None]

                def emit_chunk(k):
                    c0 = k * F
                    et = etpool.tile([NJP, F], F32, tag="et")
                    nc.tensor.matmul(et[:, :], u1_t[:, :], z1_t[:, c0:c0 + F],
                                     start=True, stop=False)
                    nc.tensor.matmul(et[:, :], u2_t[:, :], z2_t[:, c0:c0 + F],
                                     start=False, stop=True)
                    p_t = ppool.tile([NJP, F], BF16, tag="p")
                    p_tiles[k] = p_t
                    eng = nc.vector
                    # specie runs among this chunk's 4 nodes
                    i = 0
                    while i < 4:
                        j = i
                        s = slot_species[4 * k + i]
                        while j < 4 and slot_species[4 * k + j] == s:
                            j += 1
                        w_ap = _bcast_free(wt_t[:, s * C:(s + 1) * C], j - i, C)
                        eng.tensor_mul(
                            p_t[:, i * C:j * C].rearrange("j (n c) -> j n c", c=C),
                            et[:, i * C:j * C].rearrange("j (n c) -> j n c", c=C),
                            w_ap,
                        )
                        i = j

                def emit_selk(k):
                    p_t = p_tiles.pop(k)
                    for jj in range(4):
                        t = 4 * k + jj
                        if t % GN == 0:
                            yps_box[0] = ypspool.tile([C, GN * YW], F32,
                                                      name="yps", tag="yps")
                        yps = yps_box[0]
                        nc.tensor.matmul(
                            yps[:, (t % GN) * YW:(t % GN + 1) * YW],
                            p_t[:, jj * C:(jj + 1) * C],
                            sel_t[:, :],
                            start=True, stop=True,
                        )
                        if t % GN == GN - 1 or t == T4 - 1:
                            g0 = (t // GN) * GN
                            cnt = t - g0 + 1
                            nc.scalar.copy(
                                out=y_all[:, g0 * YW:(g0 + cnt) * YW],
                                in_=yps[:, :cnt * YW],
                            )

                def emit_phase4(h):
                    n0 = h * 128
                    gn = min(128, T4 - n0)
                    ot = opool.tile([128, 1152], F32, tag="ot")
                    for l in range(3):
                        ml = MS[l]
                        otv = ot[0:gn, OBASE[l]:OBASE[l] + 128 * ml].rearrange(
                            "n (d x) -> n x d", x=ml)
                        for m in range(ml):
                            ops = opspool.tile([128, C], F32, tag="ops")
                            nc.tensor.matmul(
                                ops[0:gn, :],
                                yv[:, LOFF[l] + m, n0:n0 + gn],
                                wl_t[:, l * C:(l + 1) * C],
                                start=True, stop=True,
                            )
                            nc.scalar.copy(out=otv[:, m, :], in_=ops[0:gn, :])
                    nc.sync.dma_start(out=OUT[n0:n0 + gn, :], in_=ot[0:gn, :])

                done_h = [0]

                def maybe_phase4(k_done):
                    # group h complete after selk of chunk (h+1)*32 - 1
                    while done_h[0] < NGF and k_done == (done_h[0] + 1) * 32 - 1:
                        emit_phase4(done_h[0])
                        done_h[0] += 1

                for k in range(NCH):
                    emit_chunk(k)
                    if k >= LAG:
                        emit_selk(k - LAG)
                        maybe_phase4(k - LAG)
                for k in range(max(0, NCH - LAG), NCH):
                    emit_selk(k)
                    maybe_phase4(k)
                for h in range(done_h[0], (T4 + 127) // 128):
                    emit_phase4(h)
    return nc


_CACHE = {}


def _get_program(slot_species, repeat=1):
    key = (tuple(slot_species), repeat)
    if key not in _CACHE:
        _CACHE[key] = _build_program(slot_species, repeat)
    return _CACHE[key]


def make_in_maps(inputs):
    """Host prep shared by kernel() and the timing harness."""
    slot_species, slots = _layout(inputs["node_specie"])
    z1s, z2s, U1, U2, WT, SEL, WL = _host_pack(inputs, slots)
    in_maps = [
        {"Z1": z1s[i], "Z2": z2s[i], "U1": U1, "U2": U2,
         "WT": WT, "SEL": SEL, "WL": WL}
        for i in range(NCORES)
    ]
    return slot_species, slots, in_maps


def assemble(results, slots):
    out = np.zeros((N_NODES, 1152), np.float32)
    for i in range(NCORES):
        co = results[i]["OUT"]
        sl = slots[i]
        valid = sl >= 0
        out[sl[valid]] = co[valid]
    return out


def kernel(**inputs):
    inputs = {k: np.asarray(v) for k, v in inputs.items()}
    slot_species, slots, in_maps = make_in_maps(inputs)
    nc = _get_program(slot_species)
    res = run_bass_kernel_spmd(nc, in_maps, list(range(NCORES)))
    return assemble(res.results, slots)
